# revision 6
# baseline (speedup 1.0000x reference)
"""CARC attention kernel v3 for 8 Trainium2 NeuronCores.

Sharding: 4 query-blocks x 2 head-groups (core = hg*4 + qb).  Each core
computes its 5 heads for its 1024 queries and emits a PARTIAL output;
the host adds the two head-groups' partials per query block.

Per-core hT is pre-ROTATED on the host so this core's query block is
always columns 0-1023 (self-attention is key-order invariant), making
the compiled module identical across cores and dropping the separate
query staging buffer.

Phase split (keeps the PE HAM clock-gate warm and avoids FIFO stalls):
  Phase A: background-source attention for all (head, query-chunk)
    passes.  Needs only the small bg K/V DMAs up front.  The hidden
    DMA+cast slabs and ALL self K/V projections weave through phase A's
    PE slack.  Partial ctx (with its denominator row) parks in SBUF
    as bf16.  PV runs fp8e4 DoubleRow (bg probs fit fp8 range).
  Phase B: self-source attention, everything resident; probs bf16
    (self scores reach ~9.4 so probs hit ~e^7.4).  Normalize adds the
    phase-A partial back in, then out-projection weaves into later
    chunks.
PSUM: 6 banks score ring (2 x [128,3,512]) + 1 bank ctx + 1 bank misc.
exp bias -2 (softmax-invariant) keeps fp8 probs in range.
"""

import numpy as np

import concourse.bass as bass
import concourse.mybir as mybir
import concourse.tile as tile

F32 = mybir.dt.float32
BF16 = mybir.dt.bfloat16
FP8 = mybir.dt.float8e4
AF = mybir.ActivationFunctionType
DR = mybir.MatmulPerfMode.DoubleRow

B, L, C = 1, 4096, 640
H, D = 10, 64
ALPHA = 0.42
N_CORES = 8
SCALE = 1.0 / np.sqrt(D)

NQB = 4
NHG = 2
HC = H // NHG
Q = L // NQB
NPAIR = (HC + 1) // 2
N_CC = C // 128
NKT_SRC = L // 128
VSTRIDE = 80  # fp8 bg V arena per-head stride (64 v + 1 ones + pad to 16B)
VSB = 66      # bf16 self V arena per-head stride (64 v + 1 ones + pad)

CHUNKS = [(3 * i, 3) for i in range(10)] + [(30, 2)]
KPROJ_GATE = 32  # earliest phase-A chunk index for self K/V projection weave


def emit(nc: bass.Bass):
    hT = nc.declare_dram_parameter("hT", [C, L], F32, isOutput=False)
    kbgT = nc.declare_dram_parameter("KbgT", [HC, D, L], F32, isOutput=False)
    vbg = nc.declare_dram_parameter("Vbg", [HC, L, D], F32, isOutput=False)
    wq = nc.declare_dram_parameter("Wq", [C, HC * D], F32, isOutput=False)
    wk = nc.declare_dram_parameter("Wk", [C, HC * D], F32, isOutput=False)
    wv = nc.declare_dram_parameter("Wv", [C, HC * D], F32, isOutput=False)
    wob = nc.declare_dram_parameter("WoB", [HC, D + 1, C], F32, isOutput=False)
    out = nc.declare_dram_parameter("out", [Q, C], F32, isOutput=True)

    with tile.TileContext(nc) as tc:
        with (
            tc.tile_pool(name="singles", bufs=1) as singles,
            tc.tile_pool(name="stage", bufs=2) as stage,
            tc.tile_pool(name="bgstage", bufs=2) as bgstage,
            tc.tile_pool(name="probs", bufs=3) as probs_pool,
            tc.tile_pool(name="fin", bufs=2) as fin_pool,
            tc.tile_pool(name="outsb", bufs=2) as outsb_pool,
            tc.tile_pool(name="ps_sc", bufs=2, space="PSUM") as ps_sc,
            tc.tile_pool(name="ps_ctx", bufs=1, space="PSUM") as ps_ctx,
            tc.tile_pool(name="ps_misc", bufs=1, space="PSUM") as ps_misc,
        ):
            hT_bf = singles.tile([128, N_CC, L], BF16, tag="hT_bf")
            wq_bf = singles.tile([128, N_CC, HC * D], BF16, tag="wq_bf")
            wk_bf = singles.tile([128, N_CC, HC * D], BF16, tag="wk_bf")
            wv_bf = singles.tile([128, N_CC, HC * D], BF16, tag="wv_bf")
            wob_bf = singles.tile([D + 1, HC, C], BF16, tag="wob_bf")
            qT_bf = singles.tile([128, NPAIR, Q], BF16, tag="qT_bf")
            kT_bf = singles.tile([128, NPAIR, 2 * L], BF16, tag="kT_bf")
            varena = singles.tile([128, NKT_SRC, HC * VSTRIDE], FP8, tag="va")
            vself = singles.tile([128, NKT_SRC, HC * VSB], BF16, tag="vs")
            ctxT = singles.tile([D + 1, HC, Q], BF16, tag="ctxT")
            ctxbg = singles.tile([D + 1, 2 * HC, 512], BF16, tag="ctxbg")
            ones65 = singles.tile([D + 1, D], BF16, tag="ones65")
            nc.vector.memset(ones65, 1.0)
            bneg = singles.tile([128, 1], F32, tag="bneg")
            nc.vector.memset(bneg, -2.0)
            dscr = singles.tile([128, 1], BF16, tag="dscr")
            for h in range(HC):
                nc.vector.memset(
                    varena[:, :, VSTRIDE * h + D : VSTRIDE * h + D + 1], 1.0
                )
                nc.vector.memset(
                    vself[:, :, VSB * h + D : VSB * h + D + 1], 1.0
                )

            # ---- item emitters -------------------------------------------
            def ht_item(s):
                st = stage.tile([128, N_CC, 256], F32, tag="stage")
                for i in range(N_CC):
                    nc.sync.dma_start(
                        out=st[:, i, :],
                        in_=hT[128 * i : 128 * (i + 1), 256 * s : 256 * (s + 1)],
                    )
                nc.vector.tensor_copy(
                    out=hT_bf[:, :, 256 * s : 256 * (s + 1)], in_=st
                )

            def qproj_item(qs, g):
                M = min(128, HC * D - 128 * g)
                ps = ps_misc.tile([128, 512], F32, tag="mi", name=f"qp{qs}{g}")
                for i in range(N_CC):
                    nc.tensor.matmul(
                        ps[0:M, :],
                        lhsT=wq_bf[:, i, 128 * g : 128 * g + M],
                        rhs=hT_bf[:, i, 512 * qs : 512 * (qs + 1)],
                        start=(i == 0),
                        stop=(i == N_CC - 1),
                    )
                nc.vector.tensor_copy(
                    out=qT_bf[0:M, g, 512 * qs : 512 * (qs + 1)], in_=ps[0:M, :]
                )

            def kproj_item(g, s):
                M = min(128, HC * D - 128 * g)
                ps = ps_misc.tile([128, 512], F32, tag="mi", name=f"kp{g}{s}")
                for i in range(N_CC):
                    nc.tensor.matmul(
                        ps[0:M, :],
                        lhsT=wk_bf[:, i, 128 * g : 128 * g + M],
                        rhs=hT_bf[:, i, 512 * s : 512 * (s + 1)],
                        start=(i == 0),
                        stop=(i == N_CC - 1),
                    )
                nc.vector.tensor_copy(
                    out=kT_bf[0:M, g, 512 * s : 512 * (s + 1)], in_=ps[0:M, :]
                )

            def vproj_item(kt):
                ps = ps_misc.tile([128, 512], F32, tag="mi", name=f"vp{kt}")
                for i in range(N_CC):
                    nc.tensor.matmul(
                        ps[:, 0 : HC * D],
                        lhsT=hT_bf[:, i, 128 * kt : 128 * (kt + 1)],
                        rhs=wv_bf[:, i, :],
                        start=(i == 0),
                        stop=(i == N_CC - 1),
                    )
                nc.vector.tensor_copy(
                    out=vself[:, kt, :].rearrange("p (h e) -> p h e", e=VSB)[
                        :, :, 0:D
                    ],
                    in_=ps[:, 0 : HC * D].rearrange("p (h d) -> p h d", d=D),
                )

            def bgk_item(g, sl):  # 512-key slabs, 8 per pair group
                st = bgstage.tile([128, 512], F32, tag="kbg", name=f"bk{g}{sl}")
                for half in range(2):
                    h = 2 * g + half
                    if h >= HC:
                        continue
                    nc.sync.dma_start(
                        out=st[64 * half : 64 * half + 64, :],
                        in_=kbgT[h, :, 512 * sl : 512 * (sl + 1)],
                    )
                M = 128 if 2 * g + 1 < HC else 64
                nc.vector.tensor_copy(
                    out=kT_bf[0:M, g, L + 512 * sl : L + 512 * (sl + 1)],
                    in_=st[0:M, :],
                )

            def bgv_item(h, sl):  # 1024-key slabs, 4 per head
                st = bgstage.tile([128, 8, D], F32, tag="vbg", name=f"bv{h}{sl}")
                nc.sync.dma_start(
                    out=st,
                    in_=vbg[h, 1024 * sl : 1024 * (sl + 1), :].rearrange(
                        "(kt q) d -> q kt d", q=128
                    ),
                )
                nc.vector.tensor_scalar_mul(
                    varena[:, 8 * sl : 8 * (sl + 1), VSTRIDE * h : VSTRIDE * h + D],
                    st,
                    ALPHA,
                )

            def outproj_item(qt):
                for n0, nw in ((0, 512), (512, 128)):
                    ps = ps_misc.tile([128, 512], F32, tag="mi", name=f"op{qt}{n0}")
                    for h in range(HC):
                        nc.tensor.matmul(
                            ps[:, 0:nw],
                            lhsT=ctxT[:, h, 128 * qt : 128 * (qt + 1)],
                            rhs=wob_bf[:, h, n0 : n0 + nw],
                            start=(h == 0),
                            stop=(h == HC - 1),
                        )
                    o_sb = outsb_pool.tile(
                        [128, 512], F32, tag="o_sb", name=f"ob{qt}{n0}"
                    )
                    nc.vector.tensor_copy(out=o_sb[:, 0:nw], in_=ps[:, 0:nw])
                    nc.sync.dma_start(
                        out=out[128 * qt : 128 * (qt + 1), n0 : n0 + nw],
                        in_=o_sb[:, 0:nw],
                    )

            # ---- startup -------------------------------------------------
            for half in range(2):
                n0 = 160 * half
                st = stage.tile([128, N_CC, 160], F32, tag="stage")
                nc.sync.dma_start(
                    out=st,
                    in_=wq.rearrange("(i p) n -> p i n", p=128)[:, :, n0 : n0 + 160],
                )
                nc.vector.tensor_copy(out=wq_bf[:, :, n0 : n0 + 160], in_=st)
            for s in range(4):
                ht_item(s)
            qproj_item(0, 0)
            qproj_item(1, 0)
            bgk_item(0, 0)
            bgv_item(0, 0)
            bgv_item(1, 0)
            def w_item(w_dram, w_sb, half):
                n0 = 160 * half
                st = stage.tile([128, N_CC, 160], F32, tag="stage")
                nc.sync.dma_start(
                    out=st,
                    in_=w_dram.rearrange("(i p) n -> p i n", p=128)[
                        :, :, n0 : n0 + 160
                    ],
                )
                nc.vector.tensor_copy(out=w_sb[:, :, n0 : n0 + 160], in_=st)

            def wob_item(quar):
                n0 = 160 * quar
                st = stage.tile([D + 1, HC, 160], F32, tag="stage")
                nc.sync.dma_start(
                    out=st,
                    in_=wob[:, :, n0 : n0 + 160].rearrange("h p n -> p h n"),
                )
                nc.vector.tensor_copy(out=wob_bf[:, :, n0 : n0 + 160], in_=st)

            # ---- weave queues -------------------------------------------
            kdone = [0, 0, 0]
            vdone = [0]
            bgkdone = [1, 0, 0]
            bgvdone = [1, 1] + [0] * (HC - 2)
            qdone = {(0, 0), (1, 0)}
            htdone = [4]

            def emit_item(item):
                kind = item[0]
                if kind == "ht":
                    ht_item(item[1])
                    htdone[0] = max(htdone[0], item[1] + 1)
                elif kind == "qp":
                    qproj_item(item[1], item[2])
                    qdone.add((item[1], item[2]))
                elif kind == "k":
                    kproj_item(item[1], item[2])
                    kdone[item[1]] = max(kdone[item[1]], item[2] + 1)
                elif kind == "v":
                    vproj_item(item[1])
                    vdone[0] = max(vdone[0], item[1] + 1)
                elif kind == "bk":
                    bgk_item(item[1], item[2])
                    bgkdone[item[1]] = max(bgkdone[item[1]], item[2] + 1)
                elif kind == "bv":
                    bgv_item(item[1], item[2])
                    bgvdone[item[1]] = max(bgvdone[item[1]], item[2] + 1)
                elif kind == "wk":
                    w_item(wk, wk_bf, item[1])
                elif kind == "wv":
                    w_item(wv, wv_bf, item[1])
                elif kind == "wob":
                    wob_item(item[1])
                elif kind == "op":
                    outproj_item(item[1])

            # pair-0 bg slabs first (phase A consumes them immediately),
            # then weights + hT slabs in DMA-arrival order, then the rest
            early_q = []
            for sl in range(1, 8):
                early_q.append(("bk", 0, sl))
                if sl in (1, 3, 5):
                    early_q.append(("bv", 0, (sl + 1) // 2))
                    early_q.append(("bv", 1, (sl + 1) // 2))
            mids = [("wk", 0), ("wk", 1), ("wv", 0), ("wv", 1)]
            mids += [("ht", s) for s in range(4, 16)]
            mids += [("qp", 0, 1), ("qp", 1, 1), ("qp", 0, 2), ("qp", 1, 2)]
            mids += [("wob", quar) for quar in range(4)]
            g12 = []
            for g in (1, 2):
                for sl in range(8):
                    g12.append(("bk", g, sl))
                    if sl % 2 == 0:
                        for h in (2 * g, 2 * g + 1):
                            if h < HC:
                                g12.append(("bv", h, sl // 2))
            while mids or g12:
                if mids:
                    early_q.append(mids.pop(0))
                if g12:
                    early_q.append(g12.pop(0))
            late_q = [("k", 0, s) for s in range(8)]
            late_q += [("v", kt) for kt in range(NKT_SRC)]
            late_q += [("k", g, s) for g in (1, 2) for s in range(8)]

            def drain_early(pred):
                while not pred():
                    assert early_q, "early queue exhausted"
                    emit_item(early_q.pop(0))

            # ---- phase A: background attention --------------------------
            chunk_idx = 0
            dummy_idx = [0]
            since_defib = [0]

            def defib():
                # Deliberate PE-queue bubble: the HAM clock-gate only
                # re-warms on an idle->busy transition; without these the
                # PE can stay stuck at 1.2GHz for 100+us while 100% busy.
                # A ScalarE Copy (in the exp table set, no reload) gates a
                # tiny matmul, so the PE drains and idles until ScalarE
                # catches up (~1.5-3us), then bursts again.
                since_defib[0] += 1
                if since_defib[0] >= 16:
                    since_defib[0] = 0
                    dummy_idx[0] += 1
                    nc.scalar.copy(out=dscr, in_=bneg)
                    dm = ps_misc.tile(
                        [128, 512], F32, tag="mi", name=f"df{dummy_idx[0]}"
                    )
                    nc.tensor.matmul(
                        dm[0:1, 0:16],
                        lhsT=dscr,
                        rhs=qT_bf[:, 0, 0:16],
                        start=True,
                        stop=True,
                    )
            for h in range(HC):
                g, half = h // 2, h % 2
                p0, p1 = 64 * half, 64 * half + 64
                drain_early(lambda: (0, g) in qdone and (1, g) in qdone)
                for qc in range(2):
                    ctx = ps_ctx.tile(
                        [D + 1, 512], F32, tag="ctx", name=f"cb{h}{qc}"
                    )
                    first_mm = True
                    for start_t, nt in CHUNKS:
                        hi = start_t + nt
                        drain_early(lambda: bgkdone[g] >= min((hi + 3) // 4, 8))
                        drain_early(lambda: bgvdone[h] >= min((hi + 7) // 8, 4))
                        scs = ps_sc.tile(
                            [128, 3, 512], F32, tag="sc", name=f"sA{h}{qc}{start_t}"
                        )
                        for j in range(nt):
                            kcol = L + 128 * (start_t + j)
                            nc.tensor.matmul(
                                scs[:, j, :],
                                lhsT=kT_bf[p0:p1, g, kcol : kcol + 128],
                                rhs=qT_bf[p0:p1, g, 512 * qc : 512 * (qc + 1)],
                                start=True,
                                stop=True,
                            )
                        pr = probs_pool.tile(
                            [128, 3, 512], FP8, tag="pr8", name=f"pA{h}{qc}{start_t}"
                        )
                        nc.scalar.activation(
                            pr[:, 0:nt, :], scs[:, 0:nt, :], AF.Exp,
                            scale=SCALE * ALPHA, bias=bneg,
                        )
                        nc.tensor.matmul(
                            ctx,
                            lhsT=varena[
                                :, start_t : start_t + 2,
                                VSTRIDE * h : VSTRIDE * h + D + 1,
                            ],
                            rhs=pr[:, 0:2, :],
                            perf_mode=DR,
                            start=first_mm,
                            stop=(nt == 2),
                        )
                        first_mm = False
                        if nt == 3:
                            nc.tensor.matmul(
                                ctx,
                                lhsT=varena[
                                    :, start_t + 2,
                                    VSTRIDE * h : VSTRIDE * h + D + 1,
                                ],
                                rhs=pr[:, 2, :],
                                start=False,
                                stop=False,
                            )
                        # weave: DMA/cast item every chunk; PE item when
                        # gated; ballast matmul otherwise (keeps the HAM
                        # clock-gate warm through the low-density window)
                        if early_q:
                            emit_item(early_q.pop(0))
                        if late_q and chunk_idx >= KPROJ_GATE:
                            emit_item(late_q.pop(0))
                        else:
                            dummy_idx[0] += 1
                            dm = ps_misc.tile(
                                [128, 512], F32, tag="mi",
                                name=f"dm{dummy_idx[0]}",
                            )
                            nc.tensor.matmul(
                                dm[0:D, 0:384],
                                lhsT=ones65[0:D, :],
                                rhs=qT_bf[0:D, 0, 0:384],
                                start=True,
                                stop=True,
                            )
                        defib()
                        chunk_idx += 1
                    nc.vector.tensor_copy(
                        out=ctxbg[:, 2 * h + qc, :], in_=ctx
                    )

            # ---- phase B: self attention --------------------------------
            while early_q:
                emit_item(early_q.pop(0))
            while late_q:
                emit_item(late_q.pop(0))
            weave_b = []
            pending_norm = [None]

            def norm_tail(cs, h, qc):
                """Deferred normalize: everything after ctx left PSUM."""
                def run():
                    dn = fin_pool.tile(
                        [D + 1, 512], BF16, tag="dn", name=f"d{qc}{h}"
                    )
                    nc.vector.tensor_copy(
                        out=dn[D : D + 1, :], in_=cs[D : D + 1, :]
                    )
                    bc = ps_misc.tile([128, 512], F32, tag="mi", name=f"bc{qc}{h}")
                    nc.tensor.matmul(
                        bc[0:D, :],
                        lhsT=ones65[D : D + 1, :],
                        rhs=dn[D : D + 1, :],
                        start=True,
                        stop=True,
                        tile_position=(D, 0),
                    )
                    rec = fin_pool.tile([D, 512], BF16, tag="rec", name=f"r{qc}{h}")
                    with nc.allow_low_precision(
                        reason="softmax denom reciprocal; bf16 noise ok"
                    ):
                        nc.vector.reciprocal(rec, bc[0:D, :])
                    nc.vector.tensor_mul(
                        ctxT[0:D, h, 512 * qc : 512 * (qc + 1)], cs[0:D, :], rec
                    )
                    nc.vector.memset(
                        ctxT[D : D + 1, h, 512 * qc : 512 * (qc + 1)], 1.0
                    )
                return run

            for qc in range(2):
                for h in range(HC):
                    g, half = h // 2, h % 2
                    p0, p1 = 64 * half, 64 * half + 64
                    ctx = ps_ctx.tile(
                        [D + 1, 512], F32, tag="ctx", name=f"cs{qc}{h}"
                    )
                    first_mm = True
                    ch_j = 0
                    for start_t, nt in CHUNKS:
                        scs = ps_sc.tile(
                            [128, 3, 512], F32, tag="sc", name=f"sB{qc}{h}{start_t}"
                        )
                        for j in range(nt):
                            kcol = 128 * (start_t + j)
                            nc.tensor.matmul(
                                scs[:, j, :],
                                lhsT=kT_bf[p0:p1, g, kcol : kcol + 128],
                                rhs=qT_bf[p0:p1, g, 512 * qc : 512 * (qc + 1)],
                                start=True,
                                stop=True,
                            )
                        pr = probs_pool.tile(
                            [128, 3, 512], BF16, tag="prb", name=f"pB{qc}{h}{start_t}"
                        )
                        nc.scalar.activation(
                            pr[:, 0:nt, :], scs[:, 0:nt, :], AF.Exp,
                            scale=SCALE, bias=bneg,
                        )
                        last_chunk = start_t == 30
                        for j in range(nt):
                            nc.tensor.matmul(
                                ctx,
                                lhsT=vself[
                                    :, start_t + j, VSB * h : VSB * h + D + 1
                                ],
                                rhs=pr[:, j, :],
                                start=(first_mm and j == 0),
                                stop=(last_chunk and j == nt - 1),
                            )
                        first_mm = False
                        # run the previous pass's deferred normalize once
                        # this pass is underway; out-proj weave only after
                        if ch_j == 1 and pending_norm[0] is not None:
                            pending_norm[0]()
                            pending_norm[0] = None
                        elif weave_b and pending_norm[0] is None:
                            emit_item(weave_b.pop(0))
                        defib()
                        ch_j += 1
                    # fold in phase-A partial, moving ctx PSUM -> SBUF
                    # (one op; releases the 1-deep ctx ring immediately)
                    cs = fin_pool.tile([D + 1, 512], F32, tag="cs", name=f"cs{qc}{h}")
                    nc.vector.tensor_add(cs, ctx, ctxbg[:, 2 * h + qc, :])
                    pending_norm[0] = norm_tail(cs, h, qc)
                if qc == 0:
                    weave_b = [("op", qt) for qt in range(4)]
                else:
                    pending_norm[0]()
                    pending_norm[0] = None
                    while weave_b:
                        emit_item(weave_b.pop(0))
                    for qt in range(4, 8):
                        outproj_item(qt)
    return nc


def split_waits(nc, limit=1):
    cnt = 0
    for f in nc.m.functions:
        for bb in f.blocks:
            fixed = []
            for inst in bb.instructions:
                si = inst.sync_info
                if si is not None and len(si.on_wait) > limit:
                    waits = list(si.on_wait)
                    extra, keep = waits[:-limit], waits[-limit:]
                    for w in extra:
                        cnt += 1
                        ev = mybir.InstEventSemaphore(
                            name=f"I-waitsplit-{cnt}", ins=[], outs=[]
                        )
                        ev.engine = inst.engine
                        ev.sync_info = mybir.SyncInfo(on_wait=[w], on_update=[])
                        nc.register_instruction(ev)
                        fixed.append(ev)
                    si.on_wait = keep
                fixed.append(inst)
            bb.instructions[:] = fixed
    return cnt


def build_bass():
    nc = bass.Bass()
    emit(nc)
    split_waits(nc)
    return nc


def shard_of_core(c):
    return c // NQB, c % NQB


def make_in_maps(hidden_states, K_bg, V_bg, Wq, Wk, Wv, Wo, bo):
    hT = np.ascontiguousarray(np.asarray(hidden_states, np.float32)[0].T)
    KbgT = np.ascontiguousarray(np.asarray(K_bg, np.float32).transpose(0, 2, 1))
    Vbg = np.ascontiguousarray(np.asarray(V_bg, np.float32))
    Wq = np.asarray(Wq, np.float32)
    Wk = np.asarray(Wk, np.float32)
    Wv = np.asarray(Wv, np.float32)
    Wo = np.asarray(Wo, np.float32)
    bo = np.asarray(bo, np.float32)

    per_hg = []
    for hg in range(NHG):
        cols = slice(HC * D * hg, HC * D * (hg + 1))
        wob5 = np.zeros((HC, D + 1, C), np.float32)
        wob5[:, :D, :] = Wo[cols].reshape(HC, D, C)
        if hg == 0:
            wob5[0, D, :] = bo
        per_hg.append(
            {
                "KbgT": np.ascontiguousarray(KbgT[HC * hg : HC * (hg + 1)]),
                "Vbg": np.ascontiguousarray(Vbg[HC * hg : HC * (hg + 1)]),
                "Wq": np.ascontiguousarray(Wq[:, cols]),
                "Wk": np.ascontiguousarray(Wk[:, cols]),
                "Wv": np.ascontiguousarray(Wv[:, cols]),
                "WoB": wob5,
            }
        )
    # per-qb rotated hT: query block at columns 0-1023
    hT_rot = [
        np.ascontiguousarray(np.concatenate([hT[:, Q * qb :], hT[:, : Q * qb]], 1))
        for qb in range(NQB)
    ]
    maps = []
    for c in range(N_CORES):
        hg, qb = shard_of_core(c)
        maps.append(dict(per_hg[hg], hT=hT_rot[qb]))
    return maps


_NC_CACHE = {}


def assemble(results):
    out = np.zeros((L, C), np.float32)
    for c in range(N_CORES):
        hg, qb = shard_of_core(c)
        out[Q * qb : Q * (qb + 1)] += results[c]["out"]
    return out.reshape(B, L, C)


def kernel(hidden_states, K_bg, V_bg, Wq, Wk, Wv, Wo, bo):
    if "nc" not in _NC_CACHE:
        _NC_CACHE["nc"] = build_bass()
    nc = _NC_CACHE["nc"]
    in_maps = make_in_maps(hidden_states, K_bg, V_bg, Wq, Wk, Wv, Wo, bo)
    from concourse import bass2jax

    results = bass2jax.run_bass_via_pjrt(nc, in_maps, n_cores=N_CORES)
    return assemble(results)


# revision 8
# speedup vs baseline: 1.3909x; 1.3909x over previous
"""CARC attention kernel v3 for 8 Trainium2 NeuronCores.

Sharding: 4 query-blocks x 2 head-groups (core = hg*4 + qb).  Each core
computes its 5 heads for its 1024 queries and emits a PARTIAL output;
the host adds the two head-groups' partials per query block.

Per-core hT is pre-ROTATED on the host so this core's query block is
always columns 0-1023 (self-attention is key-order invariant), making
the compiled module identical across cores and dropping the separate
query staging buffer.

Phase split (keeps the PE HAM clock-gate warm and avoids FIFO stalls):
  Phase A: background-source attention for all (head, query-chunk)
    passes.  Needs only the small bg K/V DMAs up front.  The hidden
    DMA+cast slabs and ALL self K/V projections weave through phase A's
    PE slack.  Partial ctx (with its denominator row) parks in SBUF
    as bf16.  PV runs fp8e4 DoubleRow (bg probs fit fp8 range).
  Phase B: self-source attention, everything resident; probs bf16
    (self scores reach ~9.4 so probs hit ~e^7.4).  Normalize adds the
    phase-A partial back in, then out-projection weaves into later
    chunks.
PSUM: 6 banks score ring (2 x [128,3,512]) + 1 bank ctx + 1 bank misc.
exp bias -2 (softmax-invariant) keeps fp8 probs in range.
"""

import numpy as np

import concourse.bass as bass
import concourse.mybir as mybir
import concourse.tile as tile

F32 = mybir.dt.float32
BF16 = mybir.dt.bfloat16
FP8 = mybir.dt.float8e4
AF = mybir.ActivationFunctionType
DR = mybir.MatmulPerfMode.DoubleRow

B, L, C = 1, 4096, 640
H, D = 10, 64
ALPHA = 0.42
N_CORES = 8
SCALE = 1.0 / np.sqrt(D)

NQB = 4
NHG = 2
HC = H // NHG
Q = L // NQB
NPAIR = (HC + 1) // 2
N_CC = C // 128
NKT_SRC = L // 128
VSTRIDE = 80  # fp8 bg V arena per-head stride (64 v + 1 ones + pad to 16B)
VSB = 66      # bf16 self V arena per-head stride (64 v + 1 ones + pad)

CHUNKS = [(3 * i, 3) for i in range(10)] + [(30, 2)]
KPROJ_GATE = 32  # earliest phase-A chunk index for self K/V projection weave


def emit(nc: bass.Bass):
    hT = nc.declare_dram_parameter("hT", [C, L], F32, isOutput=False)
    kbgT = nc.declare_dram_parameter("KbgT", [HC, D, L], F32, isOutput=False)
    vbg = nc.declare_dram_parameter("Vbg", [HC, L, D], F32, isOutput=False)
    wq = nc.declare_dram_parameter("Wq", [C, HC * D], F32, isOutput=False)
    wk = nc.declare_dram_parameter("Wk", [C, HC * D], F32, isOutput=False)
    wv = nc.declare_dram_parameter("Wv", [C, HC * D], F32, isOutput=False)
    wob = nc.declare_dram_parameter("WoB", [HC, D + 1, C], F32, isOutput=False)
    out = nc.declare_dram_parameter("out", [Q, C], F32, isOutput=True)

    with tile.TileContext(nc) as tc:
        with (
            tc.tile_pool(name="singles", bufs=1) as singles,
            tc.tile_pool(name="stage", bufs=2) as stage,
            tc.tile_pool(name="bgstage", bufs=2) as bgstage,
            tc.tile_pool(name="probs", bufs=3) as probs_pool,
            tc.tile_pool(name="fin", bufs=2) as fin_pool,
            tc.tile_pool(name="outsb", bufs=2) as outsb_pool,
            tc.tile_pool(name="ps_sc", bufs=2, space="PSUM") as ps_sc,
            tc.tile_pool(name="ps_ctx", bufs=1, space="PSUM") as ps_ctx,
            tc.tile_pool(name="ps_misc", bufs=1, space="PSUM") as ps_misc,
        ):
            hT_bf = singles.tile([128, N_CC, L], BF16, tag="hT_bf")
            wq_bf = singles.tile([128, N_CC, HC * D], BF16, tag="wq_bf")
            wk_bf = singles.tile([128, N_CC, HC * D], BF16, tag="wk_bf")
            wv_bf = singles.tile([128, N_CC, HC * D], BF16, tag="wv_bf")
            wob_bf = singles.tile([D + 1, HC, C], BF16, tag="wob_bf")
            qT_bf = singles.tile([128, NPAIR, Q], BF16, tag="qT_bf")
            kT_bf = singles.tile([128, NPAIR, 2 * L], BF16, tag="kT_bf")
            varena = singles.tile([128, NKT_SRC, HC * VSTRIDE], FP8, tag="va")
            vself = singles.tile([128, NKT_SRC, HC * VSB], BF16, tag="vs")
            ctxT = singles.tile([D + 1, HC, Q], BF16, tag="ctxT")
            ctxbg = singles.tile([D + 1, 2 * HC, 512], BF16, tag="ctxbg")
            ones65 = singles.tile([D + 1, D], BF16, tag="ones65")
            nc.vector.memset(ones65, 1.0)
            bneg = singles.tile([128, 1], F32, tag="bneg")
            nc.vector.memset(bneg, -2.0)
            for h in range(HC):
                nc.vector.memset(
                    varena[:, :, VSTRIDE * h + D : VSTRIDE * h + D + 1], 1.0
                )
                nc.vector.memset(
                    vself[:, :, VSB * h + D : VSB * h + D + 1], 1.0
                )

            # ---- item emitters -------------------------------------------
            def ht_item(s):
                st = stage.tile([128, N_CC, 256], F32, tag="stage")
                for i in range(N_CC):
                    nc.sync.dma_start(
                        out=st[:, i, :],
                        in_=hT[128 * i : 128 * (i + 1), 256 * s : 256 * (s + 1)],
                    )
                nc.vector.tensor_copy(
                    out=hT_bf[:, :, 256 * s : 256 * (s + 1)], in_=st
                )

            def qproj_item(qs, g):
                M = min(128, HC * D - 128 * g)
                ps = ps_misc.tile([128, 512], F32, tag="mi", name=f"qp{qs}{g}")
                for i in range(N_CC):
                    nc.tensor.matmul(
                        ps[0:M, :],
                        lhsT=wq_bf[:, i, 128 * g : 128 * g + M],
                        rhs=hT_bf[:, i, 512 * qs : 512 * (qs + 1)],
                        start=(i == 0),
                        stop=(i == N_CC - 1),
                    )
                nc.vector.tensor_copy(
                    out=qT_bf[0:M, g, 512 * qs : 512 * (qs + 1)], in_=ps[0:M, :]
                )

            def kproj_item(g, s):
                M = min(128, HC * D - 128 * g)
                ps = ps_misc.tile([128, 512], F32, tag="mi", name=f"kp{g}{s}")
                for i in range(N_CC):
                    nc.tensor.matmul(
                        ps[0:M, :],
                        lhsT=wk_bf[:, i, 128 * g : 128 * g + M],
                        rhs=hT_bf[:, i, 512 * s : 512 * (s + 1)],
                        start=(i == 0),
                        stop=(i == N_CC - 1),
                    )
                nc.vector.tensor_copy(
                    out=kT_bf[0:M, g, 512 * s : 512 * (s + 1)], in_=ps[0:M, :]
                )

            def vproj_item(kt):
                ps = ps_misc.tile([128, 512], F32, tag="mi", name=f"vp{kt}")
                for i in range(N_CC):
                    nc.tensor.matmul(
                        ps[:, 0 : HC * D],
                        lhsT=hT_bf[:, i, 128 * kt : 128 * (kt + 1)],
                        rhs=wv_bf[:, i, :],
                        start=(i == 0),
                        stop=(i == N_CC - 1),
                    )
                nc.vector.tensor_copy(
                    out=vself[:, kt, :].rearrange("p (h e) -> p h e", e=VSB)[
                        :, :, 0:D
                    ],
                    in_=ps[:, 0 : HC * D].rearrange("p (h d) -> p h d", d=D),
                )

            def bgk_item(g, sl):  # 512-key slabs, 8 per pair group
                st = bgstage.tile([128, 512], F32, tag="kbg", name=f"bk{g}{sl}")
                for half in range(2):
                    h = 2 * g + half
                    if h >= HC:
                        continue
                    nc.sync.dma_start(
                        out=st[64 * half : 64 * half + 64, :],
                        in_=kbgT[h, :, 512 * sl : 512 * (sl + 1)],
                    )
                M = 128 if 2 * g + 1 < HC else 64
                nc.vector.tensor_copy(
                    out=kT_bf[0:M, g, L + 512 * sl : L + 512 * (sl + 1)],
                    in_=st[0:M, :],
                )

            def bgv_item(h, sl):  # 1024-key slabs, 4 per head
                st = bgstage.tile([128, 8, D], F32, tag="vbg", name=f"bv{h}{sl}")
                nc.sync.dma_start(
                    out=st,
                    in_=vbg[h, 1024 * sl : 1024 * (sl + 1), :].rearrange(
                        "(kt q) d -> q kt d", q=128
                    ),
                )
                nc.vector.tensor_scalar_mul(
                    varena[:, 8 * sl : 8 * (sl + 1), VSTRIDE * h : VSTRIDE * h + D],
                    st,
                    ALPHA,
                )

            def outproj_item(qt):
                for n0, nw in ((0, 512), (512, 128)):
                    ps = ps_misc.tile([128, 512], F32, tag="mi", name=f"op{qt}{n0}")
                    for h in range(HC):
                        nc.tensor.matmul(
                            ps[:, 0:nw],
                            lhsT=ctxT[:, h, 128 * qt : 128 * (qt + 1)],
                            rhs=wob_bf[:, h, n0 : n0 + nw],
                            start=(h == 0),
                            stop=(h == HC - 1),
                        )
                    o_sb = outsb_pool.tile(
                        [128, 512], F32, tag="o_sb", name=f"ob{qt}{n0}"
                    )
                    nc.vector.tensor_copy(out=o_sb[:, 0:nw], in_=ps[:, 0:nw])
                    nc.sync.dma_start(
                        out=out[128 * qt : 128 * (qt + 1), n0 : n0 + nw],
                        in_=o_sb[:, 0:nw],
                    )

            # ---- startup -------------------------------------------------
            for half in range(2):
                n0 = 160 * half
                st = stage.tile([128, N_CC, 160], F32, tag="stage")
                nc.sync.dma_start(
                    out=st,
                    in_=wq.rearrange("(i p) n -> p i n", p=128)[:, :, n0 : n0 + 160],
                )
                nc.vector.tensor_copy(out=wq_bf[:, :, n0 : n0 + 160], in_=st)
            for s in range(4):
                ht_item(s)
            qproj_item(0, 0)
            qproj_item(1, 0)
            bgk_item(0, 0)
            bgv_item(0, 0)
            bgv_item(1, 0)
            def w_item(w_dram, w_sb, half):
                n0 = 160 * half
                st = stage.tile([128, N_CC, 160], F32, tag="stage")
                nc.sync.dma_start(
                    out=st,
                    in_=w_dram.rearrange("(i p) n -> p i n", p=128)[
                        :, :, n0 : n0 + 160
                    ],
                )
                nc.vector.tensor_copy(out=w_sb[:, :, n0 : n0 + 160], in_=st)

            def wob_item(quar):
                n0 = 160 * quar
                st = stage.tile([D + 1, HC, 160], F32, tag="stage")
                nc.sync.dma_start(
                    out=st,
                    in_=wob[:, :, n0 : n0 + 160].rearrange("h p n -> p h n"),
                )
                nc.vector.tensor_copy(out=wob_bf[:, :, n0 : n0 + 160], in_=st)

            # ---- weave queues -------------------------------------------
            kdone = [0, 0, 0]
            vdone = [0]
            bgkdone = [1, 0, 0]
            bgvdone = [1, 1] + [0] * (HC - 2)
            qdone = {(0, 0), (1, 0)}
            htdone = [4]

            def emit_item(item):
                kind = item[0]
                if kind == "ht":
                    ht_item(item[1])
                    htdone[0] = max(htdone[0], item[1] + 1)
                elif kind == "qp":
                    qproj_item(item[1], item[2])
                    qdone.add((item[1], item[2]))
                elif kind == "k":
                    kproj_item(item[1], item[2])
                    kdone[item[1]] = max(kdone[item[1]], item[2] + 1)
                elif kind == "v":
                    vproj_item(item[1])
                    vdone[0] = max(vdone[0], item[1] + 1)
                elif kind == "bk":
                    bgk_item(item[1], item[2])
                    bgkdone[item[1]] = max(bgkdone[item[1]], item[2] + 1)
                elif kind == "bv":
                    bgv_item(item[1], item[2])
                    bgvdone[item[1]] = max(bgvdone[item[1]], item[2] + 1)
                elif kind == "wk":
                    w_item(wk, wk_bf, item[1])
                elif kind == "wv":
                    w_item(wv, wv_bf, item[1])
                elif kind == "wob":
                    wob_item(item[1])
                elif kind == "op":
                    outproj_item(item[1])

            # pair-0 bg slabs first (phase A consumes them immediately),
            # then weights + hT slabs in DMA-arrival order, then the rest
            early_q = []
            for sl in range(1, 8):
                early_q.append(("bk", 0, sl))
                if sl in (1, 3, 5):
                    early_q.append(("bv", 0, (sl + 1) // 2))
                    early_q.append(("bv", 1, (sl + 1) // 2))
            mids = [("wk", 0), ("wk", 1), ("wv", 0), ("wv", 1)]
            mids += [("ht", s) for s in range(4, 16)]
            mids += [("qp", 0, 1), ("qp", 1, 1), ("qp", 0, 2), ("qp", 1, 2)]
            mids += [("wob", quar) for quar in range(4)]
            g12 = []
            for g in (1, 2):
                for sl in range(8):
                    g12.append(("bk", g, sl))
                    if sl % 2 == 0:
                        for h in (2 * g, 2 * g + 1):
                            if h < HC:
                                g12.append(("bv", h, sl // 2))
            while mids or g12:
                if mids:
                    early_q.append(mids.pop(0))
                if g12:
                    early_q.append(g12.pop(0))
            late_q = [("k", 0, s) for s in range(8)]
            late_q += [("v", kt) for kt in range(NKT_SRC)]
            late_q += [("k", g, s) for g in (1, 2) for s in range(8)]

            def drain_early(pred):
                while not pred():
                    assert early_q, "early queue exhausted"
                    emit_item(early_q.pop(0))

            # ---- phase A: background attention --------------------------
            chunk_idx = 0
            for h in range(HC):
                g, half = h // 2, h % 2
                p0, p1 = 64 * half, 64 * half + 64
                drain_early(lambda: (0, g) in qdone and (1, g) in qdone)
                for qc in range(2):
                    ctx = ps_ctx.tile(
                        [D + 1, 512], F32, tag="ctx", name=f"cb{h}{qc}"
                    )
                    first_mm = True
                    for start_t, nt in CHUNKS:
                        hi = start_t + nt
                        drain_early(lambda: bgkdone[g] >= min((hi + 3) // 4, 8))
                        drain_early(lambda: bgvdone[h] >= min((hi + 7) // 8, 4))
                        scs = ps_sc.tile(
                            [128, 3, 512], F32, tag="sc", name=f"sA{h}{qc}{start_t}"
                        )
                        for j in range(nt):
                            kcol = L + 128 * (start_t + j)
                            nc.tensor.matmul(
                                scs[:, j, :],
                                lhsT=kT_bf[p0:p1, g, kcol : kcol + 128],
                                rhs=qT_bf[p0:p1, g, 512 * qc : 512 * (qc + 1)],
                                start=True,
                                stop=True,
                            )
                        pr = probs_pool.tile(
                            [128, 3, 512], FP8, tag="pr8", name=f"pA{h}{qc}{start_t}"
                        )
                        nc.scalar.activation(
                            pr[:, 0:nt, :], scs[:, 0:nt, :], AF.Exp,
                            scale=SCALE * ALPHA, bias=bneg,
                        )
                        nc.tensor.matmul(
                            ctx,
                            lhsT=varena[
                                :, start_t : start_t + 2,
                                VSTRIDE * h : VSTRIDE * h + D + 1,
                            ],
                            rhs=pr[:, 0:2, :],
                            perf_mode=DR,
                            start=first_mm,
                            stop=(nt == 2),
                        )
                        first_mm = False
                        if nt == 3:
                            nc.tensor.matmul(
                                ctx,
                                lhsT=varena[
                                    :, start_t + 2,
                                    VSTRIDE * h : VSTRIDE * h + D + 1,
                                ],
                                rhs=pr[:, 2, :],
                                start=False,
                                stop=False,
                            )
                        # weave: DMA/cast item every chunk; PE item when gated
                        if early_q:
                            emit_item(early_q.pop(0))
                        if late_q and chunk_idx >= KPROJ_GATE:
                            emit_item(late_q.pop(0))
                        chunk_idx += 1
                    nc.vector.tensor_copy(
                        out=ctxbg[:, 2 * h + qc, :], in_=ctx
                    )

            # ---- phase B: self attention --------------------------------
            while early_q:
                emit_item(early_q.pop(0))
            while late_q:
                emit_item(late_q.pop(0))
            weave_b = []
            for qc in range(2):
                for h in range(HC):
                    g, half = h // 2, h % 2
                    p0, p1 = 64 * half, 64 * half + 64
                    ctx = ps_ctx.tile(
                        [D + 1, 512], F32, tag="ctx", name=f"cs{qc}{h}"
                    )
                    first_mm = True
                    for start_t, nt in CHUNKS:
                        scs = ps_sc.tile(
                            [128, 3, 512], F32, tag="sc", name=f"sB{qc}{h}{start_t}"
                        )
                        for j in range(nt):
                            kcol = 128 * (start_t + j)
                            nc.tensor.matmul(
                                scs[:, j, :],
                                lhsT=kT_bf[p0:p1, g, kcol : kcol + 128],
                                rhs=qT_bf[p0:p1, g, 512 * qc : 512 * (qc + 1)],
                                start=True,
                                stop=True,
                            )
                        pr = probs_pool.tile(
                            [128, 3, 512], BF16, tag="prb", name=f"pB{qc}{h}{start_t}"
                        )
                        nc.scalar.activation(
                            pr[:, 0:nt, :], scs[:, 0:nt, :], AF.Exp,
                            scale=SCALE, bias=bneg,
                        )
                        last_chunk = start_t == 30
                        for j in range(nt):
                            nc.tensor.matmul(
                                ctx,
                                lhsT=vself[
                                    :, start_t + j, VSB * h : VSB * h + D + 1
                                ],
                                rhs=pr[:, j, :],
                                start=(first_mm and j == 0),
                                stop=(last_chunk and j == nt - 1),
                            )
                        first_mm = False
                        if weave_b:
                            emit_item(weave_b.pop(0))
                    # normalize: fold in phase-A partial, then 1/denom
                    nc.vector.tensor_add(
                        ctx, ctx, ctxbg[:, 2 * h + qc, :]
                    )
                    dn = fin_pool.tile([D + 1, 512], BF16, tag="dn", name=f"d{qc}{h}")
                    nc.vector.tensor_copy(out=dn[D : D + 1, :], in_=ctx[D : D + 1, :])
                    bc = ps_misc.tile([128, 512], F32, tag="mi", name=f"bc{qc}{h}")
                    nc.tensor.matmul(
                        bc[0:D, :],
                        lhsT=ones65[D : D + 1, :],
                        rhs=dn[D : D + 1, :],
                        start=True,
                        stop=True,
                        tile_position=(D, 0),
                    )
                    rec = fin_pool.tile([D, 512], BF16, tag="rec", name=f"r{qc}{h}")
                    with nc.allow_low_precision(
                        reason="softmax denom reciprocal; bf16 noise ok"
                    ):
                        nc.vector.reciprocal(rec, bc[0:D, :])
                    nc.vector.tensor_mul(
                        ctxT[0:D, h, 512 * qc : 512 * (qc + 1)], ctx[0:D, :], rec
                    )
                    nc.vector.memset(
                        ctxT[D : D + 1, h, 512 * qc : 512 * (qc + 1)], 1.0
                    )
                if qc == 0:
                    weave_b = [("op", qt) for qt in range(4)]
                else:
                    while weave_b:
                        emit_item(weave_b.pop(0))
                    for qt in range(4, 8):
                        outproj_item(qt)
    return nc


def split_waits(nc, limit=1):
    cnt = 0
    for f in nc.m.functions:
        for bb in f.blocks:
            fixed = []
            for inst in bb.instructions:
                si = inst.sync_info
                if si is not None and len(si.on_wait) > limit:
                    waits = list(si.on_wait)
                    extra, keep = waits[:-limit], waits[-limit:]
                    for w in extra:
                        cnt += 1
                        ev = mybir.InstEventSemaphore(
                            name=f"I-waitsplit-{cnt}", ins=[], outs=[]
                        )
                        ev.engine = inst.engine
                        ev.sync_info = mybir.SyncInfo(on_wait=[w], on_update=[])
                        nc.register_instruction(ev)
                        fixed.append(ev)
                    si.on_wait = keep
                fixed.append(inst)
            bb.instructions[:] = fixed
    return cnt


def build_bass():
    nc = bass.Bass()
    emit(nc)
    split_waits(nc)
    return nc


def shard_of_core(c):
    return c // NQB, c % NQB


def make_in_maps(hidden_states, K_bg, V_bg, Wq, Wk, Wv, Wo, bo):
    hT = np.ascontiguousarray(np.asarray(hidden_states, np.float32)[0].T)
    KbgT = np.ascontiguousarray(np.asarray(K_bg, np.float32).transpose(0, 2, 1))
    Vbg = np.ascontiguousarray(np.asarray(V_bg, np.float32))
    Wq = np.asarray(Wq, np.float32)
    Wk = np.asarray(Wk, np.float32)
    Wv = np.asarray(Wv, np.float32)
    Wo = np.asarray(Wo, np.float32)
    bo = np.asarray(bo, np.float32)

    per_hg = []
    for hg in range(NHG):
        cols = slice(HC * D * hg, HC * D * (hg + 1))
        wob5 = np.zeros((HC, D + 1, C), np.float32)
        wob5[:, :D, :] = Wo[cols].reshape(HC, D, C)
        if hg == 0:
            wob5[0, D, :] = bo
        per_hg.append(
            {
                "KbgT": np.ascontiguousarray(KbgT[HC * hg : HC * (hg + 1)]),
                "Vbg": np.ascontiguousarray(Vbg[HC * hg : HC * (hg + 1)]),
                "Wq": np.ascontiguousarray(Wq[:, cols]),
                "Wk": np.ascontiguousarray(Wk[:, cols]),
                "Wv": np.ascontiguousarray(Wv[:, cols]),
                "WoB": wob5,
            }
        )
    # per-qb rotated hT: query block at columns 0-1023
    hT_rot = [
        np.ascontiguousarray(np.concatenate([hT[:, Q * qb :], hT[:, : Q * qb]], 1))
        for qb in range(NQB)
    ]
    maps = []
    for c in range(N_CORES):
        hg, qb = shard_of_core(c)
        maps.append(dict(per_hg[hg], hT=hT_rot[qb]))
    return maps


_NC_CACHE = {}


def assemble(results):
    out = np.zeros((L, C), np.float32)
    for c in range(N_CORES):
        hg, qb = shard_of_core(c)
        out[Q * qb : Q * (qb + 1)] += results[c]["out"]
    return out.reshape(B, L, C)


def kernel(hidden_states, K_bg, V_bg, Wq, Wk, Wv, Wo, bo):
    if "nc" not in _NC_CACHE:
        _NC_CACHE["nc"] = build_bass()
    nc = _NC_CACHE["nc"]
    in_maps = make_in_maps(hidden_states, K_bg, V_bg, Wq, Wk, Wv, Wo, bo)
    from concourse import bass2jax

    results = bass2jax.run_bass_via_pjrt(nc, in_maps, n_cores=N_CORES)
    return assemble(results)
